# revision 2
# baseline (speedup 1.0000x reference)
"""Causal attention head (k==v source quirk) on 8 trn2 NeuronCores — v2.

Math per batch b:
  q = x[b] @ WQ ; kv = x[b] @ WV        (k and v are the SAME projection)
  S = q @ kv^T ; causal mask ; P = softmax(S) (no sqrt(d) scale)
  out[b] = P @ kv

Sharding: core = (b, h). KEY-PARITY split: core h owns key tiles
{t : t % 2 == h} (16 tiles of 128 keys) and processes ALL 4096 queries
of its batch, producing a partial numerator + denominator; the host
combines the two cores of a batch in float64:
  out = (numA + numB) / (denA + denB).

SPMD uniformity: the host permutes x columns so each core's buffer is
[own k0, peer k0, own k1, peer k1, ...] (identity for h=0, adjacent
128-col swap for h=1). Query buffer tile p then always attends its own
key tiles 0..floor(p/2); the last slot is the diagonal tile
(compile-time triangular affine_select) when p is even, and a
full-or-dead tile (per-core 1/0 flag multiply) when p is odd.

Engine-cost-driven layout (matmul cost ~ out free size x cycles/row;
fp32=4, fp16=1, fp16-transpose=1; stationary-operand loads are free):
 - projections in NATURAL orientation: out [128 tok, kv|q] with the x
   tile as the stationary operand (8x fewer rows than score-major
   orientation would need); V-natural falls out for free.
 - QK via an fp16 hi/lo split in TWO matmuls per slot, sharing one
   stationary operand, using partition stacking:
     st = [kh;kl]^T.[ql;qh] + [kh;kl]^T.[qh;ql]
        = kh.qh + kh.ql + kl.qh + kl.ql  (the COMPLETE product set)
   The stacks are built by single fp16 transposes of naturally-split
   halves placed side by side — no cross-partition engine copies.
 - PV with pt as the stationary operand: out [128 q, 65] so N=65 per
   128x128 cell. The ones column of V' accumulates the softmax
   denominator; the divide happens on the host in float64.

PSUM discipline: a matmul with start=True zeroes its whole 2KB bank on
the partitions it writes, so every concurrently-open accumulation (two
proj buffers, two transpose staging tiles, two score slots, even/odd PV
accumulators) owns a full bank: 8 banks exactly. The even/odd PV
accumulations of a pair are phase-SERIALIZED (e then o) and drained
interleaved into the NEXT pair's QK stream (2 ops per slot), which also
keeps the PE fed while exp chases the scores. pt tiles are a 32-deep
ring so a pair's probabilities survive until its PV drains.
"""

import os
import sys

import numpy as np

sys.path.insert(0, "/opt/trn_rl_repo")

import concourse.bass as bass
import concourse.bacc as bacc
import concourse.mybir as mybir
from concourse.bass_utils import run_bass_kernel_spmd
from concourse.tile import TileContext

P = 128
T = 4096
C = 1024
D = 64
NCT = C // P          # 8 contraction tiles
NPIECE = 4            # x pieces of 1024 tokens
NT = T // P           # 32 token tiles (= query tiles per core)
NK = NT // 2          # 16 own key tiles per core
NPAIR = NT // 2       # 16 query-tile pairs

F32 = mybir.dt.float32
F16 = mybir.dt.float16

PROJ_FP32 = True  # fp32 x/w + fp32 proj matmuls (more accurate, +10us)


def build_nc():
    nc = bacc.Bacc("TRN2")
    # PROJ_FP32: x/w ship fp32 and projections are single fp32 matmul chains
    # (matches the fp32 reference noise level). The fp16 alternative (hi/lo
    # pairs, 3 fp16 passes) is ~6us faster but measured ~2x the error vs the
    # fp32 reference — too close to the 2e-2 gate to ship.
    XDT = F32 if PROJ_FP32 else F16
    xth = nc.dram_tensor("xth", [C, T], XDT, kind="ExternalInput")
    wvqh = nc.dram_tensor("wvqh", [C, 2 * D], XDT, kind="ExternalInput")
    if not PROJ_FP32:
        xtl = nc.dram_tensor("xtl", [C, T], F16, kind="ExternalInput")
        wvql = nc.dram_tensor("wvql", [C, 2 * D], F16, kind="ExternalInput")
    flg_d = nc.dram_tensor("flg", [P, 1], F32, kind="ExternalInput")
    # flat SBUF mirror layout: o[q, p*(D+1)+d] = out[p*128+q, d]; the host
    # untangles it. Keeps the output DMA at 128 fat descriptors.
    o = nc.dram_tensor("o", [P, NT * (D + 1)], F32, kind="ExternalOutput")

    with TileContext(nc) as tc:
        with (
            tc.tile_pool(name="persist", bufs=1) as persist,
            tc.tile_pool(name="xpool", bufs=3) as xpool,
            tc.tile_pool(name="natpool", bufs=6) as natpool,
            tc.tile_pool(name="ptpool", bufs=32) as ptpool,
            tc.tile_pool(name="pproj", bufs=2, space="PSUM") as pproj,
            tc.tile_pool(name="ptrp", bufs=2, space="PSUM") as ptrp,
            tc.tile_pool(name="pst", bufs=2, space="PSUM") as pst,
            tc.tile_pool(name="pacc", bufs=2, space="PSUM") as pacc,
        ):
            # --- constants ---
            ident16 = persist.tile([P, P], F16, tag="id16", name="id16")
            nc.vector.memset(ident16, 1.0)
            nc.gpsimd.affine_select(
                out=ident16, in_=ident16, pattern=[[-1, P]],
                compare_op=mybir.AluOpType.is_equal, fill=0.0,
                base=0, channel_multiplier=1,
            )
            # weight DMA split so the first proj matmul (needs only e=0,1)
            # isn't gated on the full weight transfer
            wh_sb = persist.tile([P, NCT, 2 * D], XDT, tag="wh", name="wh")
            nc.sync.dma_start(
                wh_sb[:, 0:2, :],
                wvqh[0 : 2 * P, :].rearrange("(j p) d -> p j d", p=P),
            )
            if not PROJ_FP32:
                wl_sb = persist.tile([P, NCT, 2 * D], F16, tag="wl", name="wl")

            # --- persistent SBUF state ---
            khl = persist.tile([P, NK * P], F16, tag="khl", name="khl")
            qlh = persist.tile([P, T], F16, tag="qlh", name="qlh")
            qhl = persist.tile([P, T], F16, tag="qhl", name="qhl")
            vp = persist.tile([P, NK, D + 1], F32, tag="vp", name="vp")
            nc.vector.memset(vp[:, :, D : D + 1], 1.0)
            o_sb = persist.tile([P, NT, D + 1], F32, tag="o_sb", name="o_sb")

            pending = []  # deferred PV/evac ops from the previous pair

            def drain(n):
                for _ in range(min(n, len(pending))):
                    pending.pop(0)()

            for c in range(NPIECE):
                # ---- load x piece c: buffer cols [1024c, 1024(c+1)) ----
                xh = xpool.tile([P, NCT, 1024], XDT, tag="xh", name=f"xh_{c}")
                if not PROJ_FP32:
                    xl = xpool.tile([P, NCT, 1024], F16, tag="xl", name=f"xl_{c}")
                # column-slab DMAs: one instruction covers all 8 c-tiles of a
                # token range (HWDGE charges ~625ns per DMA instruction).
                # piece 0 uses fine leading slabs so proj starts early; each
                # range ships xh then xl (the 3rd proj pass needs xl last).
                chunks = ((0, 128), (128, 256), (256, 384), (384, 512),
                          (512, 640), (640, 768), (768, 896), (896, 1024))
                xpairs = ((xh, xth),) if PROJ_FP32 else ((xh, xth), (xl, xtl))
                for ci, (lo, hi) in enumerate(chunks):
                    for xsb, xdr in xpairs:
                        if c == 0 and ci == 0:
                            # split the very first slab by c-tile halves so
                            # the first proj matmuls start ~0.7us earlier
                            for es in (slice(0, 4), slice(4, NCT)):
                                nc.sync.dma_start(
                                    xsb[:, es, lo:hi],
                                    xdr[P * es.start : P * es.stop,
                                        lo:hi].rearrange(
                                        "(j p) t -> p j t", p=P
                                    ),
                                )
                            continue
                        nc.sync.dma_start(
                            xsb[:, :, lo:hi],
                            xdr[:, 1024 * c + lo : 1024 * c + hi].rearrange(
                                "(j p) t -> p j t", p=P
                            ),
                        )
                    if c == 0 and ci == 0:
                        # rest of the weights + flags after the first slab
                        nc.sync.dma_start(
                            wh_sb[:, 2:NCT, :],
                            wvqh[2 * P :, :].rearrange("(j p) d -> p j d", p=P),
                        )
                        if not PROJ_FP32:
                            nc.sync.dma_start(
                                wl_sb,
                                wvql[:, :].rearrange("(j p) d -> p j d", p=P),
                            )
                        flg0 = persist.tile([P, 1], F32, tag="flg0", name="flg0")
                        nc.sync.dma_start(flg0, flg_d[:, :])
                        flg = persist.tile([P, 1], F32, tag="flg", name="flg")
                        nc.vector.tensor_copy(flg, flg0)
                # ---- projections + fp16 splits, 8 buffer tiles; the
                # transposes of group i are deferred into group i+1 so the
                # PE never waits on the freshly-written DVE splits ----
                deferred_tr = []

                def transposes(c, i, kj, bt, kn, qn, qn2):
                    def op():
                        trk = ptrp.tile([P, P], F16, tag="tr",
                                        name=f"trk_{c}_{i}")
                        nc.tensor.transpose(trk, kn, ident16)
                        nc.vector.tensor_copy(
                            khl[:, P * kj : P * (kj + 1)], trk)
                        drain(1)
                        for z, (qt_, bt_) in enumerate(((qn, bt), (qn2, bt + 1))):
                            qs = slice(P * bt_, P * (bt_ + 1))
                            trq = ptrp.tile([P, P], F16, tag="tr",
                                            name=f"trq_{c}_{i}_{z}")
                            nc.tensor.transpose(trq, qt_, ident16)
                            nc.vector.tensor_copy(qlh[:, qs], trq)
                            nc.vector.tensor_copy(qhl[0:D, qs],
                                                  trq[D : 2 * D, :])
                            nc.vector.tensor_copy(qhl[D : 2 * D, qs],
                                                  trq[0:D, :])
                            drain(1)
                    return op

                for i in range(4):
                    kj = 4 * c + i          # own key tile index
                    bt = 8 * c + 2 * i      # even buffer tile (own)
                    # own tile: kv|q stacked -> [128 tok, 128]; fp16 3-pass,
                    # pass-outer order so the xl passes come last
                    pp = pproj.tile([P, 2 * D], F32, tag="pp", name=f"pp_{c}_{i}")
                    passes = ((xh, wh_sb),) if PROJ_FP32 else \
                        ((xh, wh_sb), (xh, wl_sb), (xl, wh_sb))
                    NP_ = len(passes)
                    for pi, (xa, wa) in enumerate(passes):
                        for e in range(NCT):
                            nc.tensor.matmul(
                                pp,
                                xa[:, e, 256 * i : 256 * i + P],
                                wa[:, e, :],
                                start=(pi == 0 and e == 0),
                                stop=(pi == NP_ - 1 and e == NCT - 1),
                            )
                        drain(2)
                    # natural fp16 splits: [kh | kl] and [ql | qh]
                    kn = natpool.tile([P, 2 * D], F16, tag="kn", name=f"kn_{c}_{i}")
                    qn = natpool.tile([P, 2 * D], F16, tag="qn", name=f"qn_{c}_{i}")
                    nc.vector.tensor_copy(vp[:, kj, 0:D], pp[:, 0:D])
                    nc.vector.tensor_copy(kn[:, 0:D], pp[:, 0:D])      # kh
                    nc.vector.tensor_sub(kn[:, D : 2 * D], pp[:, 0:D], kn[:, 0:D])
                    nc.vector.tensor_copy(qn[:, D : 2 * D], pp[:, D : 2 * D])  # qh
                    nc.vector.tensor_sub(qn[:, 0:D], pp[:, D : 2 * D],
                                         qn[:, D : 2 * D])             # ql
                    # peer tile: q only
                    pq = pproj.tile([P, 2 * D], F32, tag="pp", name=f"pq_{c}_{i}")
                    for pi, (xa, wa) in enumerate(passes):
                        for e in range(NCT):
                            nc.tensor.matmul(
                                pq[:, 0:D],
                                xa[:, e, 256 * i + P : 256 * i + 2 * P],
                                wa[:, e, D : 2 * D],
                                start=(pi == 0 and e == 0),
                                stop=(pi == NP_ - 1 and e == NCT - 1),
                            )
                        drain(2)
                    qn2 = natpool.tile([P, 2 * D], F16, tag="qn", name=f"qn2_{c}_{i}")
                    nc.vector.tensor_copy(qn2[:, D : 2 * D], pq[:, 0:D])   # qh
                    nc.vector.tensor_sub(qn2[:, 0:D], pq[:, 0:D],
                                         qn2[:, D : 2 * D])                # ql
                    deferred_tr.append(transposes(c, i, kj, bt, kn, qn, qn2))
                    if len(deferred_tr) > 1:
                        deferred_tr.pop(0)()
                while deferred_tr:
                    deferred_tr.pop(0)()
                # ---- stream out the previous piece's finished outputs ----
                if c >= 1:
                    lo = 8 * (c - 1) * (D + 1)
                    hi = 8 * c * (D + 1)
                    nc.sync.dma_start(
                        o[:, lo:hi],
                        o_sb[:, 8 * (c - 1) : 8 * c, :],
                    )
                # ---- attention pairs of this piece ----
                for rr in range(4):
                    r = 4 * c + rr
                    last = (r == NPAIR - 1)
                    qs = slice(256 * r, 256 * (r + 1))
                    pts = []  # per slot: (pt tile, base col)
                    own = []  # last pair: its own PV, inlined trailing ~2 slots
                    for g in range(0, r + 1, 2):
                        # two score slots share one PSUM bank: slot g starts
                        # (zeroing the whole bank), slot g+1 accumulates into
                        # its untouched half, the group closes on its stop
                        s1 = min(g + 1, r)
                        w = 256 * (s1 - g + 1)
                        st = pst.tile([P, 512], F32, tag="st", name=f"st_{r}_{g}")
                        for si, s in enumerate(range(g, s1 + 1)):
                            sl = slice(256 * si, 256 * si + 256)
                            ks = slice(P * s, P * (s + 1))
                            nc.tensor.matmul(
                                st[:, sl], khl[:, ks], qlh[:, qs],
                                start=(si == 0), stop=False,
                            )
                            nc.tensor.matmul(
                                st[:, sl], khl[:, ks], qhl[:, qs],
                                start=False, stop=(s == s1),
                            )
                        pt = ptpool.tile([P, 512], F32, tag="pt",
                                         name=f"pt_{r}_{g}")
                        nc.scalar.activation(
                            pt[:, 0:w], st[:, 0:w],
                            mybir.ActivationFunctionType.Exp
                        )
                        for si in range(s1 - g + 1):
                            pts.append((pt, 256 * si))
                        if s1 == r:
                            off = 256 * (s1 - g)
                            # even query tile: diagonal -> triangular mask
                            nc.gpsimd.affine_select(
                                out=pt[:, off : off + P], in_=pt[:, off : off + P],
                                pattern=[[1, P]],
                                compare_op=mybir.AluOpType.is_ge, fill=0.0,
                                base=0, channel_multiplier=-1,
                            )
                            # odd query tile: full (flag=1) or dead (flag=0)
                            nc.gpsimd.tensor_scalar_mul(
                                pt[:, off + P : off + 2 * P],
                                pt[:, off + P : off + 2 * P], flg[:, 0:1],
                            )
                        if last:
                            if not own:
                                own = _make_pv(nc, pacc, vp, o_sb, r, pts)
                            if g >= 2:
                                for _ in range(min(4, len(own))):
                                    own.pop(0)()
                        drain(2 * (s1 - g + 1))
                    if last:
                        for op in own:
                            op()
                    else:
                        pending.extend(_make_pv(nc, pacc, vp, o_sb, r, pts))
            drain(len(pending))
            # pairs 12-14 finished during the last pair's QK stream; only
            # the last pair's 2 query tiles remain for the true tail
            nc.sync.dma_start(
                o[:, 24 * (D + 1) : 30 * (D + 1)], o_sb[:, 24:30, :]
            )
            nc.sync.dma_start(
                o[:, 30 * (D + 1) :], o_sb[:, 30:NT, :]
            )
    if not nc.is_finalized():
        nc.finalize()
    return nc


def _make_pv(nc, pacc, vp, o_sb, r, pts):
    """Deferred PV ops for pair r, interleaved [pv_e(s), pv_o(s)]... + evacs.

    acc_e / acc_o live in separate PSUM banks (pacc bufs=2), so both
    accumulation groups may be open concurrently. Closures read pts[s]
    lazily — the list is shared with the QK loop and grows as exp ops are
    emitted.
    """
    acc = {}

    def pv(s, half):
        def op():
            if s == 0:
                acc[half] = pacc.tile([128, D + 1], F32, tag="acc",
                                      name=f"acc{half}_{r}")
            pt, base = pts[s]
            nc.tensor.matmul(
                acc[half], pt[:, base + 128 * half : base + 128 * (half + 1)],
                vp[:, s, :], start=(s == 0), stop=(s == r),
            )
        return op

    ops = []
    for s in range(r + 1):
        ops.append(pv(s, 0))
        ops.append(pv(s, 1))
    ops.append(lambda: nc.vector.tensor_copy(o_sb[:, 2 * r, :], acc[0]))
    ops.append(lambda: nc.vector.tensor_copy(o_sb[:, 2 * r + 1, :], acc[1]))
    return ops


_CACHED_NC = None


def kernel(**inputs):
    global _CACHED_NC
    x = np.ascontiguousarray(np.asarray(inputs["x"], dtype=np.float32))
    WQ = np.ascontiguousarray(np.asarray(inputs["WQ"], dtype=np.float32))
    WV = np.ascontiguousarray(np.asarray(inputs["WV"], dtype=np.float32))
    B = x.shape[0]

    if _CACHED_NC is None:
        _CACHED_NC = build_nc()
    nc = _CACHED_NC

    wvq_in = np.concatenate([WV, WQ], axis=1)
    if PROJ_FP32:
        wh, wl = wvq_in.astype(np.float32), None
    else:
        wh = wvq_in.astype(np.float16)
        wl = (wvq_in - wh.astype(np.float32)).astype(np.float16)
    # h=1 swaps adjacent 128-col tile pairs so own keys sit at even slots
    swap = np.arange(T).reshape(NT // 2, 2, P)[:, ::-1, :].reshape(T)

    in_maps = []
    for core in range(8):
        b, h = divmod(core, 2)
        xtb = x[b].T  # [C, T]
        if h:
            xtb = xtb[:, swap]
        if PROJ_FP32:
            m = {
                "xth": np.ascontiguousarray(xtb.astype(np.float32)),
                "wvqh": np.ascontiguousarray(wh),
            }
        else:
            xhv = xtb.astype(np.float16)
            xlo = (xtb - xhv.astype(np.float32)).astype(np.float16)
            m = {
                "xth": np.ascontiguousarray(xhv),
                "xtl": np.ascontiguousarray(xlo),
                "wvqh": np.ascontiguousarray(wh),
                "wvql": np.ascontiguousarray(wl),
            }
        m["flg"] = np.full((P, 1), 1.0 - h, dtype=np.float32)
        in_maps.append(m)

    trace = os.environ.get("KERNEL_TRACE", "0") == "1"
    res = run_bass_kernel_spmd(nc, in_maps, core_ids=list(range(8)), trace=trace)
    kernel._last_results = res

    out = np.empty((B, T, D), dtype=np.float32)
    for b in range(B):
        # o[q, p*(D+1)+d] -> [p*128+q, d] buffer-token order
        oa, ob = (
            res.results[2 * b + h]["o"]
            .astype(np.float64)
            .reshape(P, NT, D + 1)
            .transpose(1, 0, 2)
            .reshape(T, D + 1)
            for h in (0, 1)
        )
        ob = ob.reshape(NT // 2, 2, P, D + 1)[:, ::-1].reshape(T, D + 1)
        tot = oa + ob
        out[b] = (tot[:, :D] / tot[:, D:]).astype(np.float32)
    return out


# revision 3
# speedup vs baseline: 1.0008x; 1.0008x over previous
"""Causal attention head (k==v source quirk) on 8 trn2 NeuronCores — v2.

Math per batch b:
  q = x[b] @ WQ ; kv = x[b] @ WV        (k and v are the SAME projection)
  S = q @ kv^T ; causal mask ; P = softmax(S) (no sqrt(d) scale)
  out[b] = P @ kv

Sharding: core = (b, h). KEY-PARITY split: core h owns key tiles
{t : t % 2 == h} (16 tiles of 128 keys) and processes ALL 4096 queries
of its batch, producing a partial numerator + denominator; the host
combines the two cores of a batch in float64:
  out = (numA + numB) / (denA + denB).

SPMD uniformity: the host permutes x columns so each core's buffer is
[own k0, peer k0, own k1, peer k1, ...] (identity for h=0, adjacent
128-col swap for h=1). Query buffer tile p then always attends its own
key tiles 0..floor(p/2); the last slot is the diagonal tile
(compile-time triangular affine_select) when p is even, and a
full-or-dead tile (per-core 1/0 flag multiply) when p is odd.

Engine-cost-driven layout (matmul cost ~ out free size x cycles/row;
fp32=4, fp16=1, fp16-transpose=1; stationary-operand loads are free):
 - projections in NATURAL orientation: out [128 tok, kv|q] with the x
   tile as the stationary operand (8x fewer rows than score-major
   orientation would need); V-natural falls out for free.
 - QK via an fp16 hi/lo split in TWO matmuls per slot, sharing one
   stationary operand, using partition stacking:
     st = [kh;kl]^T.[ql;qh] + [kh;kl]^T.[qh;ql]
        = kh.qh + kh.ql + kl.qh + kl.ql  (the COMPLETE product set)
   The stacks are built by single fp16 transposes of naturally-split
   halves placed side by side — no cross-partition engine copies.
 - PV with pt as the stationary operand: out [128 q, 65] so N=65 per
   128x128 cell. The ones column of V' accumulates the softmax
   denominator; the divide happens on the host in float64.

PSUM discipline: a matmul with start=True zeroes its whole 2KB bank on
the partitions it writes, so every concurrently-open accumulation (two
proj buffers, two transpose staging tiles, two score slots, even/odd PV
accumulators) owns a full bank: 8 banks exactly. The even/odd PV
accumulations of a pair are phase-SERIALIZED (e then o) and drained
interleaved into the NEXT pair's QK stream (2 ops per slot), which also
keeps the PE fed while exp chases the scores. pt tiles are a 32-deep
ring so a pair's probabilities survive until its PV drains.
"""

import os
import sys

import numpy as np

sys.path.insert(0, "/opt/trn_rl_repo")

import concourse.bass as bass
import concourse.bacc as bacc
import concourse.mybir as mybir
from concourse.bass_utils import run_bass_kernel_spmd
from concourse.tile import TileContext

P = 128
T = 4096
C = 1024
D = 64
NCT = C // P          # 8 contraction tiles
NPIECE = 4            # x pieces of 1024 tokens
NT = T // P           # 32 token tiles (= query tiles per core)
NK = NT // 2          # 16 own key tiles per core
NPAIR = NT // 2       # 16 query-tile pairs

F32 = mybir.dt.float32
F16 = mybir.dt.float16

PROJ_FP32 = True  # fp32 x/w + fp32 proj matmuls (more accurate, +10us)


def build_nc():
    nc = bacc.Bacc("TRN2")
    # PROJ_FP32: x/w ship fp32 and projections are single fp32 matmul chains
    # (matches the fp32 reference noise level). The fp16 alternative (hi/lo
    # pairs, 3 fp16 passes) is ~6us faster but measured ~2x the error vs the
    # fp32 reference — too close to the 2e-2 gate to ship.
    XDT = F32 if PROJ_FP32 else F16
    xth = nc.dram_tensor("xth", [C, T], XDT, kind="ExternalInput")
    wvqh = nc.dram_tensor("wvqh", [C, 2 * D], XDT, kind="ExternalInput")
    if not PROJ_FP32:
        xtl = nc.dram_tensor("xtl", [C, T], F16, kind="ExternalInput")
        wvql = nc.dram_tensor("wvql", [C, 2 * D], F16, kind="ExternalInput")
    flg_d = nc.dram_tensor("flg", [P, 1], F32, kind="ExternalInput")
    # flat SBUF mirror layout: o[q, p*(D+1)+d] = out[p*128+q, d]; the host
    # untangles it. Keeps the output DMA at 128 fat descriptors.
    o = nc.dram_tensor("o", [P, NT * (D + 1)], F32, kind="ExternalOutput")

    with TileContext(nc) as tc:
        with (
            tc.tile_pool(name="persist", bufs=1) as persist,
            tc.tile_pool(name="xpool", bufs=3) as xpool,
            tc.tile_pool(name="natpool", bufs=6) as natpool,
            tc.tile_pool(name="ptpool", bufs=32) as ptpool,
            tc.tile_pool(name="pproj", bufs=2, space="PSUM") as pproj,
            tc.tile_pool(name="ptrp", bufs=2, space="PSUM") as ptrp,
            tc.tile_pool(name="pst", bufs=2, space="PSUM") as pst,
            tc.tile_pool(name="pacc", bufs=2, space="PSUM") as pacc,
        ):
            # weight DMA split so the first proj matmul (needs only e=0,1)
            # isn't gated on the full weight transfer
            wh_sb = persist.tile([P, NCT, 2 * D], XDT, tag="wh", name="wh")
            nc.sync.dma_start(
                wh_sb[:, 0:2, :],
                wvqh[0 : 2 * P, :].rearrange("(j p) d -> p j d", p=P),
            )
            # --- constants ---
            ident16 = persist.tile([P, P], F16, tag="id16", name="id16")
            nc.vector.memset(ident16, 1.0)
            nc.gpsimd.affine_select(
                out=ident16, in_=ident16, pattern=[[-1, P]],
                compare_op=mybir.AluOpType.is_equal, fill=0.0,
                base=0, channel_multiplier=1,
            )
            if not PROJ_FP32:
                wl_sb = persist.tile([P, NCT, 2 * D], F16, tag="wl", name="wl")

            # --- persistent SBUF state ---
            khl = persist.tile([P, NK * P], F16, tag="khl", name="khl")
            qlh = persist.tile([P, T], F16, tag="qlh", name="qlh")
            qhl = persist.tile([P, T], F16, tag="qhl", name="qhl")
            vp = persist.tile([P, NK, D + 1], F32, tag="vp", name="vp")
            nc.vector.memset(vp[:, :, D : D + 1], 1.0)
            o_sb = persist.tile([P, NT, D + 1], F32, tag="o_sb", name="o_sb")

            pending = []  # deferred PV/evac ops from the previous pair

            def drain(n):
                for _ in range(min(n, len(pending))):
                    pending.pop(0)[1]()

            for c in range(NPIECE):
                # ---- load x piece c: buffer cols [1024c, 1024(c+1)) ----
                xh = xpool.tile([P, NCT, 1024], XDT, tag="xh", name=f"xh_{c}")
                if not PROJ_FP32:
                    xl = xpool.tile([P, NCT, 1024], F16, tag="xl", name=f"xl_{c}")
                # column-slab DMAs: one instruction covers all 8 c-tiles of a
                # token range (HWDGE charges ~625ns per DMA instruction).
                # piece 0 uses fine leading slabs so proj starts early; each
                # range ships xh then xl (the 3rd proj pass needs xl last).
                chunks = ((0, 128), (128, 256), (256, 384), (384, 512),
                          (512, 640), (640, 768), (768, 896), (896, 1024))
                xpairs = ((xh, xth),) if PROJ_FP32 else ((xh, xth), (xl, xtl))
                for ci, (lo, hi) in enumerate(chunks):
                    for xsb, xdr in xpairs:
                        if c == 0 and ci == 0:
                            # split the very first slab by c-tile halves so
                            # the first proj matmuls start ~0.7us earlier
                            for es in (slice(0, 4), slice(4, NCT)):
                                nc.sync.dma_start(
                                    xsb[:, es, lo:hi],
                                    xdr[P * es.start : P * es.stop,
                                        lo:hi].rearrange(
                                        "(j p) t -> p j t", p=P
                                    ),
                                )
                            continue
                        nc.sync.dma_start(
                            xsb[:, :, lo:hi],
                            xdr[:, 1024 * c + lo : 1024 * c + hi].rearrange(
                                "(j p) t -> p j t", p=P
                            ),
                        )
                    if c == 0 and ci == 0:
                        # rest of the weights + flags after the first slab
                        nc.sync.dma_start(
                            wh_sb[:, 2:NCT, :],
                            wvqh[2 * P :, :].rearrange("(j p) d -> p j d", p=P),
                        )
                        if not PROJ_FP32:
                            nc.sync.dma_start(
                                wl_sb,
                                wvql[:, :].rearrange("(j p) d -> p j d", p=P),
                            )
                        flg0 = persist.tile([P, 1], F32, tag="flg0", name="flg0")
                        nc.sync.dma_start(flg0, flg_d[:, :])
                        flg = persist.tile([P, 1], F32, tag="flg", name="flg")
                        nc.vector.tensor_copy(flg, flg0)
                # ---- projections + fp16 splits, 8 buffer tiles; the
                # transposes of group i are deferred into group i+1 so the
                # PE never waits on the freshly-written DVE splits ----
                deferred_tr = []

                def transposes(c, i, kj, bt, kn, qn, qn2):
                    def op():
                        trk = ptrp.tile([P, P], F16, tag="tr",
                                        name=f"trk_{c}_{i}")
                        nc.tensor.transpose(trk, kn, ident16)
                        nc.vector.tensor_copy(
                            khl[:, P * kj : P * (kj + 1)], trk)
                        drain(1)
                        for z, (qt_, bt_) in enumerate(((qn, bt), (qn2, bt + 1))):
                            qs = slice(P * bt_, P * (bt_ + 1))
                            trq = ptrp.tile([P, P], F16, tag="tr",
                                            name=f"trq_{c}_{i}_{z}")
                            nc.tensor.transpose(trq, qt_, ident16)
                            nc.vector.tensor_copy(qlh[:, qs], trq)
                            nc.vector.tensor_copy(qhl[0:D, qs],
                                                  trq[D : 2 * D, :])
                            nc.vector.tensor_copy(qhl[D : 2 * D, qs],
                                                  trq[0:D, :])
                            drain(1)
                    return op

                for i in range(4):
                    kj = 4 * c + i          # own key tile index
                    bt = 8 * c + 2 * i      # even buffer tile (own)
                    # own tile: kv|q stacked -> [128 tok, 128]; fp16 3-pass,
                    # pass-outer order so the xl passes come last
                    pp = pproj.tile([P, 2 * D], F32, tag="pp", name=f"pp_{c}_{i}")
                    passes = ((xh, wh_sb),) if PROJ_FP32 else \
                        ((xh, wh_sb), (xh, wl_sb), (xl, wh_sb))
                    NP_ = len(passes)
                    for pi, (xa, wa) in enumerate(passes):
                        for e in range(NCT):
                            nc.tensor.matmul(
                                pp,
                                xa[:, e, 256 * i : 256 * i + P],
                                wa[:, e, :],
                                start=(pi == 0 and e == 0),
                                stop=(pi == NP_ - 1 and e == NCT - 1),
                            )
                        drain(2)
                    # natural fp16 splits: [kh | kl] and [ql | qh]
                    kn = natpool.tile([P, 2 * D], F16, tag="kn", name=f"kn_{c}_{i}")
                    qn = natpool.tile([P, 2 * D], F16, tag="qn", name=f"qn_{c}_{i}")
                    nc.vector.tensor_copy(vp[:, kj, 0:D], pp[:, 0:D])
                    nc.vector.tensor_copy(kn[:, 0:D], pp[:, 0:D])      # kh
                    nc.vector.tensor_sub(kn[:, D : 2 * D], pp[:, 0:D], kn[:, 0:D])
                    nc.vector.tensor_copy(qn[:, D : 2 * D], pp[:, D : 2 * D])  # qh
                    nc.vector.tensor_sub(qn[:, 0:D], pp[:, D : 2 * D],
                                         qn[:, D : 2 * D])             # ql
                    # peer tile: q only
                    pq = pproj.tile([P, 2 * D], F32, tag="pp", name=f"pq_{c}_{i}")
                    for pi, (xa, wa) in enumerate(passes):
                        for e in range(NCT):
                            nc.tensor.matmul(
                                pq[:, 0:D],
                                xa[:, e, 256 * i + P : 256 * i + 2 * P],
                                wa[:, e, D : 2 * D],
                                start=(pi == 0 and e == 0),
                                stop=(pi == NP_ - 1 and e == NCT - 1),
                            )
                        drain(2)
                    qn2 = natpool.tile([P, 2 * D], F16, tag="qn", name=f"qn2_{c}_{i}")
                    nc.vector.tensor_copy(qn2[:, D : 2 * D], pq[:, 0:D])   # qh
                    nc.vector.tensor_sub(qn2[:, 0:D], pq[:, 0:D],
                                         qn2[:, D : 2 * D])                # ql
                    deferred_tr.append(transposes(c, i, kj, bt, kn, qn, qn2))
                    if len(deferred_tr) > 1:
                        deferred_tr.pop(0)()
                while deferred_tr:
                    deferred_tr.pop(0)()
                # ---- stream out finished pieces; deferred to piece 3 so
                # these DMAs never delay supply-critical x slabs ----
                if c == NPIECE - 1:
                    nc.sync.dma_start(o[:, 0 : 24 * (D + 1)], o_sb[:, 0:24, :])
                # ---- attention pairs of this piece ----
                for rr in range(4):
                    r = 4 * c + rr
                    last = (r == NPAIR - 1)
                    qs = slice(256 * r, 256 * (r + 1))
                    pts = []  # per slot: (pt tile, base col)
                    own = []  # last pair: its own PV, inlined trailing ~2 slots
                    for g in range(0, r + 1, 2):
                        # two score slots share one PSUM bank: slot g starts
                        # (zeroing the whole bank), slot g+1 accumulates into
                        # its untouched half, the group closes on its stop
                        s1 = min(g + 1, r)
                        w = 256 * (s1 - g + 1)
                        st = pst.tile([P, 512], F32, tag="st", name=f"st_{r}_{g}")
                        for si, s in enumerate(range(g, s1 + 1)):
                            sl = slice(256 * si, 256 * si + 256)
                            ks = slice(P * s, P * (s + 1))
                            nc.tensor.matmul(
                                st[:, sl], khl[:, ks], qlh[:, qs],
                                start=(si == 0), stop=False,
                            )
                            nc.tensor.matmul(
                                st[:, sl], khl[:, ks], qhl[:, qs],
                                start=False, stop=(s == s1),
                            )
                        pt = ptpool.tile([P, 512], F32, tag="pt",
                                         name=f"pt_{r}_{g}")
                        nc.scalar.activation(
                            pt[:, 0:w], st[:, 0:w],
                            mybir.ActivationFunctionType.Exp
                        )
                        for si in range(s1 - g + 1):
                            pts.append((pt, 256 * si))
                        if s1 == r:
                            off = 256 * (s1 - g)
                            # even query tile: diagonal -> triangular mask
                            nc.gpsimd.affine_select(
                                out=pt[:, off : off + P], in_=pt[:, off : off + P],
                                pattern=[[1, P]],
                                compare_op=mybir.AluOpType.is_ge, fill=0.0,
                                base=0, channel_multiplier=-1,
                            )
                            # odd query tile: full (flag=1) or dead (flag=0)
                            nc.gpsimd.tensor_scalar_mul(
                                pt[:, off + P : off + 2 * P],
                                pt[:, off + P : off + 2 * P], flg[:, 0:1],
                            )
                        if last:
                            if not own:
                                own = _make_pv(nc, pacc, vp, o_sb, r, pts)
                            if g >= 2:
                                budget = 4
                                while budget and own and own[0][0] <= s1:
                                    own.pop(0)[1]()
                                    budget -= 1
                        drain(2 * (s1 - g + 1))
                    if last:
                        for _, op in own:
                            op()
                    else:
                        pending.extend(_make_pv(nc, pacc, vp, o_sb, r, pts))
            drain(len(pending))
            # pairs 12-14 finished during the last pair's QK stream; only
            # the last pair's 2 query tiles remain for the true tail
            nc.sync.dma_start(
                o[:, 24 * (D + 1) : 30 * (D + 1)], o_sb[:, 24:30, :]
            )
            nc.sync.dma_start(
                o[:, 30 * (D + 1) :], o_sb[:, 30:NT, :]
            )
    if not nc.is_finalized():
        nc.finalize()
    return nc


def _make_pv(nc, pacc, vp, o_sb, r, pts):
    """Deferred PV ops for pair r, interleaved [pv_e(s), pv_o(s)]... + evacs.

    acc_e / acc_o live in separate PSUM banks (pacc bufs=2), so both
    accumulation groups may be open concurrently. Closures read pts[s]
    lazily — the list is shared with the QK loop and grows as exp ops are
    emitted.
    """
    acc = {}

    def pv(s, half):
        def op():
            if s == 0:
                acc[half] = pacc.tile([128, D + 1], F32, tag="acc",
                                      name=f"acc{half}_{r}")
            pt, base = pts[s]
            nc.tensor.matmul(
                acc[half], pt[:, base + 128 * half : base + 128 * (half + 1)],
                vp[:, s, :], start=(s == 0), stop=(s == r),
            )
        return op

    ops = []
    for s in range(r + 1):
        ops.append((s, pv(s, 0)))
        ops.append((s, pv(s, 1)))
    ops.append((r, lambda: nc.vector.tensor_copy(o_sb[:, 2 * r, :], acc[0])))
    ops.append((r, lambda: nc.vector.tensor_copy(o_sb[:, 2 * r + 1, :], acc[1])))
    return ops


_CACHED_NC = None


def kernel(**inputs):
    global _CACHED_NC
    x = np.ascontiguousarray(np.asarray(inputs["x"], dtype=np.float32))
    WQ = np.ascontiguousarray(np.asarray(inputs["WQ"], dtype=np.float32))
    WV = np.ascontiguousarray(np.asarray(inputs["WV"], dtype=np.float32))
    B = x.shape[0]

    if _CACHED_NC is None:
        _CACHED_NC = build_nc()
    nc = _CACHED_NC

    wvq_in = np.concatenate([WV, WQ], axis=1)
    if PROJ_FP32:
        wh, wl = wvq_in.astype(np.float32), None
    else:
        wh = wvq_in.astype(np.float16)
        wl = (wvq_in - wh.astype(np.float32)).astype(np.float16)
    # h=1 swaps adjacent 128-col tile pairs so own keys sit at even slots
    swap = np.arange(T).reshape(NT // 2, 2, P)[:, ::-1, :].reshape(T)

    in_maps = []
    for core in range(8):
        b, h = divmod(core, 2)
        xtb = x[b].T  # [C, T]
        if h:
            xtb = xtb[:, swap]
        if PROJ_FP32:
            m = {
                "xth": np.ascontiguousarray(xtb.astype(np.float32)),
                "wvqh": np.ascontiguousarray(wh),
            }
        else:
            xhv = xtb.astype(np.float16)
            xlo = (xtb - xhv.astype(np.float32)).astype(np.float16)
            m = {
                "xth": np.ascontiguousarray(xhv),
                "xtl": np.ascontiguousarray(xlo),
                "wvqh": np.ascontiguousarray(wh),
                "wvql": np.ascontiguousarray(wl),
            }
        m["flg"] = np.full((P, 1), 1.0 - h, dtype=np.float32)
        in_maps.append(m)

    trace = os.environ.get("KERNEL_TRACE", "0") == "1"
    res = run_bass_kernel_spmd(nc, in_maps, core_ids=list(range(8)), trace=trace)
    kernel._last_results = res

    out = np.empty((B, T, D), dtype=np.float32)
    for b in range(B):
        # o[q, p*(D+1)+d] -> [p*128+q, d] buffer-token order
        oa, ob = (
            res.results[2 * b + h]["o"]
            .astype(np.float64)
            .reshape(P, NT, D + 1)
            .transpose(1, 0, 2)
            .reshape(T, D + 1)
            for h in (0, 1)
        )
        ob = ob.reshape(NT // 2, 2, P, D + 1)[:, ::-1].reshape(T, D + 1)
        tot = oa + ob
        out[b] = (tot[:, :D] / tot[:, D:]).astype(np.float32)
    return out


# revision 4
# speedup vs baseline: 1.0192x; 1.0183x over previous
"""Causal attention head (k==v source quirk) on 8 trn2 NeuronCores — v2.

Math per batch b:
  q = x[b] @ WQ ; kv = x[b] @ WV        (k and v are the SAME projection)
  S = q @ kv^T ; causal mask ; P = softmax(S) (no sqrt(d) scale)
  out[b] = P @ kv

Sharding: core = (b, h). KEY-PARITY split: core h owns key tiles
{t : t % 2 == h} (16 tiles of 128 keys) and processes ALL 4096 queries
of its batch, producing a partial numerator + denominator; the host
combines the two cores of a batch in float64:
  out = (numA + numB) / (denA + denB).

SPMD uniformity: the host permutes x columns so each core's buffer is
[own k0, peer k0, own k1, peer k1, ...] (identity for h=0, adjacent
128-col swap for h=1). Query buffer tile p then always attends its own
key tiles 0..floor(p/2); the last slot is the diagonal tile
(compile-time triangular affine_select) when p is even, and a
full-or-dead tile (per-core 1/0 flag multiply) when p is odd.

Engine-cost-driven layout (matmul cost ~ out free size x cycles/row;
fp32=4, fp16=1, fp16-transpose=1; stationary-operand loads are free):
 - projections in NATURAL orientation: out [128 tok, kv|q] with the x
   tile as the stationary operand (8x fewer rows than score-major
   orientation would need); V-natural falls out for free.
 - QK via an fp16 hi/lo split in TWO matmuls per slot, sharing one
   stationary operand, using partition stacking:
     st = [kh;kl]^T.[ql;qh] + [kh;kl]^T.[qh;ql]
        = kh.qh + kh.ql + kl.qh + kl.ql  (the COMPLETE product set)
   The stacks are built by single fp16 transposes of naturally-split
   halves placed side by side — no cross-partition engine copies.
 - PV with pt as the stationary operand: out [128 q, 65] so N=65 per
   128x128 cell. The ones column of V' accumulates the softmax
   denominator; the divide happens on the host in float64.

PSUM discipline: a matmul with start=True zeroes its whole 2KB bank on
the partitions it writes, so every concurrently-open accumulation (two
proj buffers, two transpose staging tiles, two score slots, even/odd PV
accumulators) owns a full bank: 8 banks exactly. The even/odd PV
accumulations of a pair are phase-SERIALIZED (e then o) and drained
interleaved into the NEXT pair's QK stream (2 ops per slot), which also
keeps the PE fed while exp chases the scores. pt tiles are a 32-deep
ring so a pair's probabilities survive until its PV drains.
"""

import os
import sys

import numpy as np

sys.path.insert(0, "/opt/trn_rl_repo")

import concourse.bass as bass
import concourse.bacc as bacc
import concourse.mybir as mybir
from concourse.bass_utils import run_bass_kernel_spmd
from concourse.tile import TileContext

P = 128
T = 4096
C = 1024
D = 64
NCT = C // P          # 8 contraction tiles
NPIECE = 4            # x pieces of 1024 tokens
NT = T // P           # 32 token tiles (= query tiles per core)
NK = NT // 2          # 16 own key tiles per core
NPAIR = NT // 2       # 16 query-tile pairs

F32 = mybir.dt.float32
F16 = mybir.dt.float16

PROJ_FP32 = True  # fp32 x/w + fp32 proj matmuls (more accurate, +10us)


def build_nc():
    nc = bacc.Bacc("TRN2")
    # PROJ_FP32: x/w ship fp32 and projections are single fp32 matmul chains
    # (matches the fp32 reference noise level). The fp16 alternative (hi/lo
    # pairs, 3 fp16 passes) is ~6us faster but measured ~2x the error vs the
    # fp32 reference — too close to the 2e-2 gate to ship.
    XDT = F32 if PROJ_FP32 else F16
    xth = nc.dram_tensor("xth", [C, T], XDT, kind="ExternalInput")
    wvqh = nc.dram_tensor("wvqh", [C, 2 * D], XDT, kind="ExternalInput")
    if not PROJ_FP32:
        xtl = nc.dram_tensor("xtl", [C, T], F16, kind="ExternalInput")
        wvql = nc.dram_tensor("wvql", [C, 2 * D], F16, kind="ExternalInput")
    flg_d = nc.dram_tensor("flg", [P, 1], F32, kind="ExternalInput")
    # flat SBUF mirror layout: o[q, p*(D+1)+d] = out[p*128+q, d]; the host
    # untangles it. Keeps the output DMA at 128 fat descriptors.
    o = nc.dram_tensor("o", [P, NT * (D + 1)], F32, kind="ExternalOutput")

    with TileContext(nc) as tc:
        with (
            tc.tile_pool(name="persist", bufs=1) as persist,
            tc.tile_pool(name="xpool", bufs=3) as xpool,
            tc.tile_pool(name="natpool", bufs=6) as natpool,
            tc.tile_pool(name="ptpool", bufs=32) as ptpool,
            tc.tile_pool(name="pproj", bufs=2, space="PSUM") as pproj,
            tc.tile_pool(name="ptrp", bufs=2, space="PSUM") as ptrp,
            tc.tile_pool(name="pst", bufs=2, space="PSUM") as pst,
            tc.tile_pool(name="pacc", bufs=2, space="PSUM") as pacc,
        ):
            # weight DMA split so the first proj matmul (needs only e=0,1)
            # isn't gated on the full weight transfer
            wh_sb = persist.tile([P, NCT, 2 * D], XDT, tag="wh", name="wh")
            nc.sync.dma_start(
                wh_sb[:, 0:2, :],
                wvqh[0 : 2 * P, :].rearrange("(j p) d -> p j d", p=P),
            )
            # --- constants ---
            ident16 = persist.tile([P, P], F16, tag="id16", name="id16")
            nc.vector.memset(ident16, 1.0)
            # PE warm-up: the tensor engine ramps 0.65->2.4GHz over ~3us of
            # continuous activity. Burn the initial DMA wait on dummy
            # matmuls (results never read) so the real projections start at
            # full clock. Only the memset gates them, not the affine_select.
            for wi in range(16):
                wt = ptrp.tile([P, P], F32, tag="tr", name=f"warm_{wi}")
                nc.tensor.matmul(wt, ident16, ident16, start=True, stop=True)
            nc.gpsimd.affine_select(
                out=ident16, in_=ident16, pattern=[[-1, P]],
                compare_op=mybir.AluOpType.is_equal, fill=0.0,
                base=0, channel_multiplier=1,
            )
            if not PROJ_FP32:
                wl_sb = persist.tile([P, NCT, 2 * D], F16, tag="wl", name="wl")

            # --- persistent SBUF state ---
            khl = persist.tile([P, NK * P], F16, tag="khl", name="khl")
            qlh = persist.tile([P, T], F16, tag="qlh", name="qlh")
            qhl = persist.tile([P, T], F16, tag="qhl", name="qhl")
            vp = persist.tile([P, NK, D + 1], F32, tag="vp", name="vp")
            nc.vector.memset(vp[:, :, D : D + 1], 1.0)
            o_sb = persist.tile([P, NT, D + 1], F32, tag="o_sb", name="o_sb")

            pending = []  # deferred PV/evac ops from the previous pair

            def drain(n):
                for _ in range(min(n, len(pending))):
                    pending.pop(0)[1]()

            for c in range(NPIECE):
                # ---- load x piece c: buffer cols [1024c, 1024(c+1)) ----
                xh = xpool.tile([P, NCT, 1024], XDT, tag="xh", name=f"xh_{c}")
                if not PROJ_FP32:
                    xl = xpool.tile([P, NCT, 1024], F16, tag="xl", name=f"xl_{c}")
                # column-slab DMAs: one instruction covers all 8 c-tiles of a
                # token range (HWDGE charges ~625ns per DMA instruction).
                # piece 0 uses fine leading slabs so proj starts early; each
                # range ships xh then xl (the 3rd proj pass needs xl last).
                chunks = ((0, 128), (128, 256), (256, 384), (384, 512),
                          (512, 640), (640, 768), (768, 896), (896, 1024))
                xpairs = ((xh, xth),) if PROJ_FP32 else ((xh, xth), (xl, xtl))
                for ci, (lo, hi) in enumerate(chunks):
                    for xsb, xdr in xpairs:
                        if c == 0 and ci == 0:
                            # split the very first slab by c-tile halves so
                            # the first proj matmuls start ~0.7us earlier
                            for es in (slice(0, 4), slice(4, NCT)):
                                nc.sync.dma_start(
                                    xsb[:, es, lo:hi],
                                    xdr[P * es.start : P * es.stop,
                                        lo:hi].rearrange(
                                        "(j p) t -> p j t", p=P
                                    ),
                                )
                            continue
                        nc.sync.dma_start(
                            xsb[:, :, lo:hi],
                            xdr[:, 1024 * c + lo : 1024 * c + hi].rearrange(
                                "(j p) t -> p j t", p=P
                            ),
                        )
                    if c == 0 and ci == 0:
                        # rest of the weights + flags after the first slab
                        nc.sync.dma_start(
                            wh_sb[:, 2:NCT, :],
                            wvqh[2 * P :, :].rearrange("(j p) d -> p j d", p=P),
                        )
                        if not PROJ_FP32:
                            nc.sync.dma_start(
                                wl_sb,
                                wvql[:, :].rearrange("(j p) d -> p j d", p=P),
                            )
                        flg0 = persist.tile([P, 1], F32, tag="flg0", name="flg0")
                        nc.sync.dma_start(flg0, flg_d[:, :])
                        flg = persist.tile([P, 1], F32, tag="flg", name="flg")
                        nc.vector.tensor_copy(flg, flg0)
                # ---- projections + fp16 splits, 8 buffer tiles; the
                # transposes of group i are deferred into group i+1 so the
                # PE never waits on the freshly-written DVE splits ----
                deferred_tr = []

                def transposes(c, i, kj, bt, kn, qn, qn2):
                    def op():
                        trk = ptrp.tile([P, P], F16, tag="tr",
                                        name=f"trk_{c}_{i}")
                        nc.tensor.transpose(trk, kn, ident16)
                        nc.vector.tensor_copy(
                            khl[:, P * kj : P * (kj + 1)], trk)
                        drain(1)
                        for z, (qt_, bt_) in enumerate(((qn, bt), (qn2, bt + 1))):
                            qs = slice(P * bt_, P * (bt_ + 1))
                            trq = ptrp.tile([P, P], F16, tag="tr",
                                            name=f"trq_{c}_{i}_{z}")
                            nc.tensor.transpose(trq, qt_, ident16)
                            nc.vector.tensor_copy(qlh[:, qs], trq)
                            nc.vector.tensor_copy(qhl[0:D, qs],
                                                  trq[D : 2 * D, :])
                            nc.vector.tensor_copy(qhl[D : 2 * D, qs],
                                                  trq[0:D, :])
                            drain(1)
                    return op

                for i in range(4):
                    kj = 4 * c + i          # own key tile index
                    bt = 8 * c + 2 * i      # even buffer tile (own)
                    # own tile: kv|q stacked -> [128 tok, 128]; fp16 3-pass,
                    # pass-outer order so the xl passes come last
                    pp = pproj.tile([P, 2 * D], F32, tag="pp", name=f"pp_{c}_{i}")
                    passes = ((xh, wh_sb),) if PROJ_FP32 else \
                        ((xh, wh_sb), (xh, wl_sb), (xl, wh_sb))
                    NP_ = len(passes)
                    for pi, (xa, wa) in enumerate(passes):
                        for e in range(NCT):
                            nc.tensor.matmul(
                                pp,
                                xa[:, e, 256 * i : 256 * i + P],
                                wa[:, e, :],
                                start=(pi == 0 and e == 0),
                                stop=(pi == NP_ - 1 and e == NCT - 1),
                            )
                        drain(2)
                    # natural fp16 splits: [kh | kl] and [ql | qh]
                    kn = natpool.tile([P, 2 * D], F16, tag="kn", name=f"kn_{c}_{i}")
                    qn = natpool.tile([P, 2 * D], F16, tag="qn", name=f"qn_{c}_{i}")
                    nc.vector.tensor_copy(vp[:, kj, 0:D], pp[:, 0:D])
                    nc.vector.tensor_copy(kn[:, 0:D], pp[:, 0:D])      # kh
                    nc.vector.tensor_sub(kn[:, D : 2 * D], pp[:, 0:D], kn[:, 0:D])
                    nc.vector.tensor_copy(qn[:, D : 2 * D], pp[:, D : 2 * D])  # qh
                    nc.vector.tensor_sub(qn[:, 0:D], pp[:, D : 2 * D],
                                         qn[:, D : 2 * D])             # ql
                    # peer tile: q only
                    pq = pproj.tile([P, 2 * D], F32, tag="pp", name=f"pq_{c}_{i}")
                    for pi, (xa, wa) in enumerate(passes):
                        for e in range(NCT):
                            nc.tensor.matmul(
                                pq[:, 0:D],
                                xa[:, e, 256 * i + P : 256 * i + 2 * P],
                                wa[:, e, D : 2 * D],
                                start=(pi == 0 and e == 0),
                                stop=(pi == NP_ - 1 and e == NCT - 1),
                            )
                        drain(2)
                    qn2 = natpool.tile([P, 2 * D], F16, tag="qn", name=f"qn2_{c}_{i}")
                    nc.vector.tensor_copy(qn2[:, D : 2 * D], pq[:, 0:D])   # qh
                    nc.vector.tensor_sub(qn2[:, 0:D], pq[:, 0:D],
                                         qn2[:, D : 2 * D])                # ql
                    deferred_tr.append(transposes(c, i, kj, bt, kn, qn, qn2))
                    if len(deferred_tr) > 1:
                        deferred_tr.pop(0)()
                while deferred_tr:
                    deferred_tr.pop(0)()
                # ---- stream out finished pieces; deferred to piece 3 so
                # these DMAs never delay supply-critical x slabs ----
                if c == NPIECE - 1:
                    nc.sync.dma_start(o[:, 0 : 24 * (D + 1)], o_sb[:, 0:24, :])
                # ---- attention pairs of this piece ----
                for rr in range(4):
                    r = 4 * c + rr
                    last = (r == NPAIR - 1)
                    qs = slice(256 * r, 256 * (r + 1))
                    pts = []  # per slot: (pt tile, base col)
                    own = []  # last pair: its own PV, inlined trailing ~2 slots
                    for g in range(0, r + 1, 2):
                        # two score slots share one PSUM bank: slot g starts
                        # (zeroing the whole bank), slot g+1 accumulates into
                        # its untouched half, the group closes on its stop
                        s1 = min(g + 1, r)
                        w = 256 * (s1 - g + 1)
                        st = pst.tile([P, 512], F32, tag="st", name=f"st_{r}_{g}")
                        for si, s in enumerate(range(g, s1 + 1)):
                            sl = slice(256 * si, 256 * si + 256)
                            ks = slice(P * s, P * (s + 1))
                            nc.tensor.matmul(
                                st[:, sl], khl[:, ks], qlh[:, qs],
                                start=(si == 0), stop=False,
                            )
                            nc.tensor.matmul(
                                st[:, sl], khl[:, ks], qhl[:, qs],
                                start=False, stop=(s == s1),
                            )
                        pt = ptpool.tile([P, 512], F32, tag="pt",
                                         name=f"pt_{r}_{g}")
                        nc.scalar.activation(
                            pt[:, 0:w], st[:, 0:w],
                            mybir.ActivationFunctionType.Exp
                        )
                        for si in range(s1 - g + 1):
                            pts.append((pt, 256 * si))
                        if s1 == r:
                            off = 256 * (s1 - g)
                            # even query tile: diagonal -> triangular mask
                            nc.gpsimd.affine_select(
                                out=pt[:, off : off + P], in_=pt[:, off : off + P],
                                pattern=[[1, P]],
                                compare_op=mybir.AluOpType.is_ge, fill=0.0,
                                base=0, channel_multiplier=-1,
                            )
                            # odd query tile: full (flag=1) or dead (flag=0)
                            nc.gpsimd.tensor_scalar_mul(
                                pt[:, off + P : off + 2 * P],
                                pt[:, off + P : off + 2 * P], flg[:, 0:1],
                            )
                        if last:
                            if not own:
                                own = _make_pv(nc, pacc, vp, o_sb, r, pts)
                            if g >= 2:
                                budget = 4
                                while budget and own and own[0][0] <= s1:
                                    own.pop(0)[1]()
                                    budget -= 1
                        drain(2 * (s1 - g + 1))
                    if last:
                        for _, op in own:
                            op()
                    else:
                        pending.extend(_make_pv(nc, pacc, vp, o_sb, r, pts))
            drain(len(pending))
            # pairs 12-14 finished during the last pair's QK stream; only
            # the last pair's 2 query tiles remain for the true tail
            nc.sync.dma_start(
                o[:, 24 * (D + 1) : 30 * (D + 1)], o_sb[:, 24:30, :]
            )
            nc.sync.dma_start(
                o[:, 30 * (D + 1) :], o_sb[:, 30:NT, :]
            )
    if not nc.is_finalized():
        nc.finalize()
    return nc


def _make_pv(nc, pacc, vp, o_sb, r, pts):
    """Deferred PV ops for pair r, interleaved [pv_e(s), pv_o(s)]... + evacs.

    acc_e / acc_o live in separate PSUM banks (pacc bufs=2), so both
    accumulation groups may be open concurrently. Closures read pts[s]
    lazily — the list is shared with the QK loop and grows as exp ops are
    emitted.
    """
    acc = {}

    def pv(s, half):
        def op():
            if s == 0:
                acc[half] = pacc.tile([128, D + 1], F32, tag="acc",
                                      name=f"acc{half}_{r}")
            pt, base = pts[s]
            nc.tensor.matmul(
                acc[half], pt[:, base + 128 * half : base + 128 * (half + 1)],
                vp[:, s, :], start=(s == 0), stop=(s == r),
            )
        return op

    ops = []
    for s in range(r + 1):
        ops.append((s, pv(s, 0)))
        ops.append((s, pv(s, 1)))
    ops.append((r, lambda: nc.vector.tensor_copy(o_sb[:, 2 * r, :], acc[0])))
    ops.append((r, lambda: nc.vector.tensor_copy(o_sb[:, 2 * r + 1, :], acc[1])))
    return ops


_CACHED_NC = None


def kernel(**inputs):
    global _CACHED_NC
    x = np.ascontiguousarray(np.asarray(inputs["x"], dtype=np.float32))
    WQ = np.ascontiguousarray(np.asarray(inputs["WQ"], dtype=np.float32))
    WV = np.ascontiguousarray(np.asarray(inputs["WV"], dtype=np.float32))
    B = x.shape[0]

    if _CACHED_NC is None:
        _CACHED_NC = build_nc()
    nc = _CACHED_NC

    wvq_in = np.concatenate([WV, WQ], axis=1)
    if PROJ_FP32:
        wh, wl = wvq_in.astype(np.float32), None
    else:
        wh = wvq_in.astype(np.float16)
        wl = (wvq_in - wh.astype(np.float32)).astype(np.float16)
    # h=1 swaps adjacent 128-col tile pairs so own keys sit at even slots
    swap = np.arange(T).reshape(NT // 2, 2, P)[:, ::-1, :].reshape(T)

    in_maps = []
    for core in range(8):
        b, h = divmod(core, 2)
        xtb = x[b].T  # [C, T]
        if h:
            xtb = xtb[:, swap]
        if PROJ_FP32:
            m = {
                "xth": np.ascontiguousarray(xtb.astype(np.float32)),
                "wvqh": np.ascontiguousarray(wh),
            }
        else:
            xhv = xtb.astype(np.float16)
            xlo = (xtb - xhv.astype(np.float32)).astype(np.float16)
            m = {
                "xth": np.ascontiguousarray(xhv),
                "xtl": np.ascontiguousarray(xlo),
                "wvqh": np.ascontiguousarray(wh),
                "wvql": np.ascontiguousarray(wl),
            }
        m["flg"] = np.full((P, 1), 1.0 - h, dtype=np.float32)
        in_maps.append(m)

    trace = os.environ.get("KERNEL_TRACE", "0") == "1"
    res = run_bass_kernel_spmd(nc, in_maps, core_ids=list(range(8)), trace=trace)
    kernel._last_results = res

    out = np.empty((B, T, D), dtype=np.float32)
    for b in range(B):
        # o[q, p*(D+1)+d] -> [p*128+q, d] buffer-token order
        oa, ob = (
            res.results[2 * b + h]["o"]
            .astype(np.float64)
            .reshape(P, NT, D + 1)
            .transpose(1, 0, 2)
            .reshape(T, D + 1)
            for h in (0, 1)
        )
        ob = ob.reshape(NT // 2, 2, P, D + 1)[:, ::-1].reshape(T, D + 1)
        tot = oa + ob
        out[b] = (tot[:, :D] / tot[:, D:]).astype(np.float32)
    return out
